# revision 23
# baseline (speedup 1.0000x reference)
"""Trainium2 Bass kernel for AdaptiveReLULayer (MoE-style routed batched matmul).

    out[b] = LeakyReLU_0.2(x[b] @ weight[indices[b]] + bias)
    x: [2048, 256, 256] f32, indices: [2048] int, weight: [1024, 256, 256] f32

Strategy: data parallelism over the batch dim B=2048 across 8 NeuronCores
(256 batches/core), with an index-aware schedule: batches that share a weight
index are assigned to the same core as a "run" (length 1..4), so each run's
weight tile is DMA'd from HBM once and reused from SBUF (~47% less weight
traffic).  Run-length COUNTS are equalized across cores by splitting runs, so
all 8 cores execute the same static SPMD graph; only the data differs.

The weight gather is resolved on the host while sharding (pure addressing),
and x is pre-permuted/transposed on the host into a partition-major layout,
so every device DMA is one fully-contiguous strided block (16KB/partition
packets).  Group sizes taper at both ends (2,2,4,8,...16...,8,4,2,2) so the
first matmul starts after ~0.5MiB of loads and the final store is tiny.
Compute is bf16 with fp32 PSUM accumulation (~3e-3 relative error).
LeakyReLU: ACT copies the 512-wide PSUM bank to SBUF (bf16), then one DVE
scalar_tensor_tensor computes (v*0.2) max v in place.
"""

import numpy as np
import ml_dtypes

import concourse.bass as bass
import concourse.tile as tile
import concourse.mybir as mybir
from concourse import bacc
from concourse.bass_utils import run_bass_kernel_spmd

B, NTOK, DIN, DOUT, C = 2048, 256, 256, 256, 1024
NCORES = 8
BLOC = B // NCORES          # 256 batches per core
KC = DIN // 128             # contraction chunks of 128
TCH = NTOK // 128           # token chunks of 128
MAXRUN = 4
NEG_SLOPE = 0.2
BF16 = mybir.dt.bfloat16
F32 = mybir.dt.float32

XB = KC * NTOK              # free elems per batch of xt, per partition
WB = KC * DOUT              # free elems per run of wu, per partition
OB = TCH * DOUT             # free elems per batch of out, per partition

LAST = {}                   # stash of the last run's BassKernelResults
_CACHE = {}                 # compiled graph cache keyed by run structure


def _group_sizes(total, taper=(4, 8), mid=16):
    """Tapered group sizes: small at both ends, `mid` in the middle."""
    head = []
    need = total
    for t in taper:
        if need - t < sum(taper):
            break
        head.append(t)
        need -= t
    tail = []
    for t in taper:
        if need - t <= 0:
            break
        tail.append(t)
        need -= t
    assert need >= 0
    mids, rem = divmod(need, mid)
    out = head + [mid] * mids + ([rem] if rem else []) + tail[::-1]
    assert sum(out) == total, (out, total)
    return out


def _schedule(indices):
    """Partition the 2048 batches into 8 cores of 256 as runs of equal-index
    batches (length 1..MAXRUN).  Returns (run_lengths, perm, wu_cls): one
    shared run-length list (identical for all cores), perm [NCORES, BLOC]
    global batch ids in processing order, wu_cls [NCORES, NRUNS] weight class
    per run.
    """
    by_cls = {}
    for b, c in enumerate(indices.tolist()):
        by_cls.setdefault(c, []).append(b)

    runs = []
    for c, bs in by_cls.items():
        for i in range(0, len(bs), MAXRUN):
            runs.append((c, bs[i : i + MAXRUN]))
    runs.sort(key=lambda r: -len(r[1]))

    caps = [BLOC] * NCORES
    core_runs = [[] for _ in range(NCORES)]
    for c, bs in runs:
        while bs:
            k = int(np.argmax(caps))
            take = min(len(bs), caps[k])
            assert take > 0
            core_runs[k].append((c, bs[:take]))
            caps[k] -= take
            bs = bs[take:]
    assert all(v == 0 for v in caps)

    def counts(rl):
        n = [0] * (MAXRUN + 1)
        for c, bs in rl:
            n[len(bs)] += 1
        return n

    for L in range(MAXRUN, 1, -1):
        tgt = min(counts(rl)[L] for rl in core_runs)
        for rl in core_runs:
            while counts(rl)[L] > tgt:
                i = next(i for i, r in enumerate(rl) if len(r[1]) == L)
                c, bs = rl.pop(i)
                h = L // 2
                rl.append((c, bs[:h]))
                rl.append((c, bs[h:]))

    cn = counts(core_runs[0])
    assert all(counts(rl) == cn for rl in core_runs)

    for rl in core_runs:
        rl.sort(key=lambda r: -len(r[1]))
    run_lengths = [len(bs) for c, bs in core_runs[0]]
    perm = np.array(
        [[b for c, bs in rl for b in bs] for rl in core_runs], dtype=np.int64
    )
    wu_cls = np.array([[c for c, bs in rl] for rl in core_runs], dtype=np.int64)
    return run_lengths, perm, wu_cls


def _build(run_lengths, nonzero_bias: bool):
    nruns = len(run_lengths)
    xgs = _group_sizes(BLOC)
    wgs = _group_sizes(nruns)
    # batch j -> (x-group, index within group, group base batch)
    xg_of = []
    for gi, gs in enumerate(xgs):
        for jj in range(gs):
            xg_of.append((gi, jj))
    xg_base = np.cumsum([0] + xgs).tolist()
    wg_of = []
    for gi, gs in enumerate(wgs):
        for ss in range(gs):
            wg_of.append((gi, ss))
    wg_base = np.cumsum([0] + wgs).tolist()

    nc = bacc.Bacc(
        "TRN2", target_bir_lowering=False, debug=False, num_devices=NCORES
    )
    xt_d = nc.dram_tensor("xt", [128, BLOC * XB], BF16, kind="ExternalInput")
    wu_d = nc.dram_tensor("wu", [128, nruns * WB], BF16, kind="ExternalInput")
    bias_d = (
        nc.dram_tensor("bias", [1, DOUT], F32, kind="ExternalInput")
        if nonzero_bias
        else None
    )
    out_d = nc.dram_tensor("out", [128, BLOC * OB], BF16, kind="ExternalOutput")

    with tile.TileContext(nc) as tc:
        with (
            tc.tile_pool(name="xp", bufs=4) as xp,
            tc.tile_pool(name="io", bufs=4) as io,
            tc.tile_pool(name="wp", bufs=3) as wp,
            tc.tile_pool(name="psum", bufs=8, space=bass.MemorySpace.PSUM) as psum,
            tc.tile_pool(name="one", bufs=1) as one,
        ):
            bias_t = None
            if nonzero_bias:
                bias_t = one.tile([128, TCH, DOUT], F32, tag="bias")
                bap = bias_d.ap()
                nc.sync.dma_start(
                    out=bias_t[:],
                    in_=bass.AP(tensor=bap.tensor, offset=bap.offset,
                                ap=[[0, 128], [0, TCH], bap.ap[1]]),
                )

            xt_t = None
            out_t = None
            wu_t = None
            out_g = -1
            bp = 0
            for r, L in enumerate(run_lengths):
                wg, sw = wg_of[r]
                if sw == 0:
                    gs = wgs[wg]
                    wu_t = wp.tile([128, gs, KC, DOUT], BF16, tag="wu")
                    nc.scalar.dma_start(
                        out=wu_t[:],
                        in_=wu_d[:, wg_base[wg] * WB : wg_base[wg + 1] * WB],
                    )
                for i in range(L):
                    j = bp + i
                    g, jj = xg_of[j]
                    if jj == 0:
                        if out_t is not None:
                            nc.gpsimd.dma_start(
                                out=out_d[
                                    :, xg_base[out_g] * OB : xg_base[out_g + 1] * OB
                                ],
                                in_=out_t[:],
                            )
                        gs = xgs[g]
                        xt_t = xp.tile([128, gs, KC, NTOK], BF16, tag="xt")
                        nc.sync.dma_start(
                            out=xt_t[:],
                            in_=xt_d[:, xg_base[g] * XB : xg_base[g + 1] * XB],
                        )
                        out_t = io.tile([128, gs, TCH, DOUT], BF16, tag="out")
                        out_g = g
                    ps = psum.tile([128, TCH, DOUT], F32, tag="ps")
                    for t in range(TCH):
                        for k in range(KC):
                            nc.tensor.matmul(
                                ps[:, t, :],
                                xt_t[:, jj, k, t * 128 : (t + 1) * 128],
                                wu_t[:, sw, k, :],
                                start=(k == 0),
                                stop=(k == KC - 1),
                            )
                    if nonzero_bias:
                        tmp = io.tile([128, TCH, DOUT], F32, tag="tmp")
                        nc.vector.scalar_tensor_tensor(
                            out=tmp[:], in0=ps[:, :, :], scalar=1.0,
                            in1=bias_t[:],
                            op0=mybir.AluOpType.mult, op1=mybir.AluOpType.add,
                        )
                        nc.vector.scalar_tensor_tensor(
                            out=out_t[:, jj, :, :], in0=tmp[:],
                            scalar=NEG_SLOPE, in1=tmp[:],
                            op0=mybir.AluOpType.mult, op1=mybir.AluOpType.max,
                        )
                    else:
                        nc.scalar.activation(
                            out=out_t[:, jj, :, :], in_=ps[:, :, :],
                            func=mybir.ActivationFunctionType.Copy,
                        )
                        nc.vector.scalar_tensor_tensor(
                            out=out_t[:, jj, :, :], in0=out_t[:, jj, :, :],
                            scalar=NEG_SLOPE, in1=out_t[:, jj, :, :],
                            op0=mybir.AluOpType.mult, op1=mybir.AluOpType.max,
                        )
                bp += L
            assert bp == BLOC
            nc.gpsimd.dma_start(
                out=out_d[:, xg_base[out_g] * OB : xg_base[out_g + 1] * OB],
                in_=out_t[:],
            )
    nc.compile()
    return nc


def kernel(x, indices, weight, bias, _trace=False):
    x = np.asarray(x)
    indices = np.asarray(indices).astype(np.int64)
    weight = np.asarray(weight)
    bias = np.asarray(bias)

    run_lengths, perm, wu_cls = _schedule(indices)
    nruns = len(run_lengths)

    # x[b, n, i] at permuted b, i=(k,p)  ->  xt[c][p, j, k, n]  (partition-major)
    xb = x.astype(ml_dtypes.bfloat16)
    xt = np.ascontiguousarray(
        xb[perm.reshape(-1)]
        .reshape(NCORES, BLOC, NTOK, KC, 128)
        .transpose(0, 4, 1, 3, 2)
    ).reshape(NCORES, 128, BLOC * XB)
    # weight[cls, i, o], i=(k,p) at per-run classes -> wu[c][p, r, k, o]
    wb = weight.astype(ml_dtypes.bfloat16).reshape(C, KC, 128, DOUT)
    wu = np.ascontiguousarray(
        wb[wu_cls.reshape(-1)]
        .reshape(NCORES, nruns, KC, 128, DOUT)
        .transpose(0, 3, 1, 2, 4)
    ).reshape(NCORES, 128, nruns * WB)

    nonzero_bias = bool(np.any(bias))
    key = (tuple(run_lengths), nonzero_bias)
    nc = _CACHE.get(key)
    if nc is None:
        nc = _build(run_lengths, nonzero_bias)
        _CACHE.clear()
        _CACHE[key] = nc

    in_maps = []
    for c in range(NCORES):
        m = {"xt": xt[c], "wu": wu[c]}
        if nonzero_bias:
            m["bias"] = np.ascontiguousarray(
                bias.reshape(1, DOUT).astype(np.float32)
            )
        in_maps.append(m)

    res = run_bass_kernel_spmd(
        nc, in_maps, core_ids=list(range(NCORES)), trace=_trace
    )
    LAST["results"] = res
    LAST["nruns"] = nruns

    # out[c][p, j, t, o] -> out[perm[c, j], n=(t,p), o]
    full = np.empty((B, NTOK, DOUT), dtype=np.float32)
    for c in range(NCORES):
        o = np.asarray(res.results[c]["out"]).reshape(128, BLOC, TCH, DOUT)
        o = o.transpose(1, 2, 0, 3).reshape(BLOC, NTOK, DOUT).astype(np.float32)
        full[perm[c]] = o
    return full


# revision 24
# speedup vs baseline: 1.0787x; 1.0787x over previous
"""Trainium2 Bass kernel for AdaptiveReLULayer (MoE-style routed batched matmul).

    out[b] = LeakyReLU_0.2(x[b] @ weight[indices[b]] + bias)
    x: [2048, 256, 256] f32, indices: [2048] int, weight: [1024, 256, 256] f32

Strategy: data parallelism over the batch dim B=2048 across 8 NeuronCores
(256 batches/core), with an index-aware schedule: batches that share a weight
index are assigned to the same core as a "run" (length 1..4), so each run's
weight tile is DMA'd from HBM once and reused from SBUF (~47% less weight
traffic).  Run-length COUNTS are equalized across cores by splitting runs, so
all 8 cores execute the same static SPMD graph; only the data differs.

The weight gather is resolved on the host while sharding (pure addressing),
and x is pre-permuted/transposed on the host into a partition-major layout,
so every device DMA is one fully-contiguous strided block (16KB/partition
packets).  Group sizes taper at both ends (2,2,4,8,...16...,8,4,2,2) so the
first matmul starts after ~0.5MiB of loads and the final store is tiny.
Compute is bf16 with fp32 PSUM accumulation (~3e-3 relative error).
LeakyReLU: ACT copies the 512-wide PSUM bank to SBUF (bf16), then one DVE
scalar_tensor_tensor computes (v*0.2) max v in place.
"""

import numpy as np
import ml_dtypes

import concourse.bass as bass
import concourse.tile as tile
import concourse.mybir as mybir
from concourse import bacc
from concourse.bass_utils import run_bass_kernel_spmd

B, NTOK, DIN, DOUT, C = 2048, 256, 256, 256, 1024
NCORES = 8
BLOC = B // NCORES          # 256 batches per core
KC = DIN // 128             # contraction chunks of 128
TCH = NTOK // 128           # token chunks of 128
MAXRUN = 4
NEG_SLOPE = 0.2
BF16 = mybir.dt.bfloat16
F32 = mybir.dt.float32

XB = KC * NTOK              # free elems per batch of xt, per partition
WB = KC * DOUT              # free elems per run of wu, per partition
OB = TCH * DOUT             # free elems per batch of out, per partition

LAST = {}                   # stash of the last run's BassKernelResults
_CACHE = {}                 # compiled graph cache keyed by run structure


def _group_sizes(total, taper=(4, 8), mid=16):
    """Tapered group sizes: small at both ends, `mid` in the middle."""
    head = []
    need = total
    for t in taper:
        if need - t < sum(taper):
            break
        head.append(t)
        need -= t
    tail = []
    for t in taper:
        if need - t <= 0:
            break
        tail.append(t)
        need -= t
    assert need >= 0
    mids, rem = divmod(need, mid)
    out = head + [mid] * mids + ([rem] if rem else []) + tail[::-1]
    assert sum(out) == total, (out, total)
    return out


def _schedule(indices):
    """Partition the 2048 batches into 8 cores of 256 as runs of equal-index
    batches (length 1..MAXRUN).  Returns (run_lengths, perm, wu_cls): one
    shared run-length list (identical for all cores), perm [NCORES, BLOC]
    global batch ids in processing order, wu_cls [NCORES, NRUNS] weight class
    per run.
    """
    by_cls = {}
    for b, c in enumerate(indices.tolist()):
        by_cls.setdefault(c, []).append(b)

    runs = []
    for c, bs in by_cls.items():
        for i in range(0, len(bs), MAXRUN):
            runs.append((c, bs[i : i + MAXRUN]))
    runs.sort(key=lambda r: -len(r[1]))

    caps = [BLOC] * NCORES
    core_runs = [[] for _ in range(NCORES)]
    for c, bs in runs:
        while bs:
            k = int(np.argmax(caps))
            take = min(len(bs), caps[k])
            assert take > 0
            core_runs[k].append((c, bs[:take]))
            caps[k] -= take
            bs = bs[take:]
    assert all(v == 0 for v in caps)

    def counts(rl):
        n = [0] * (MAXRUN + 1)
        for c, bs in rl:
            n[len(bs)] += 1
        return n

    for L in range(MAXRUN, 1, -1):
        tgt = min(counts(rl)[L] for rl in core_runs)
        for rl in core_runs:
            while counts(rl)[L] > tgt:
                i = next(i for i, r in enumerate(rl) if len(r[1]) == L)
                c, bs = rl.pop(i)
                h = L // 2
                rl.append((c, bs[:h]))
                rl.append((c, bs[h:]))

    cn = counts(core_runs[0])
    assert all(counts(rl) == cn for rl in core_runs)

    for rl in core_runs:
        rl.sort(key=lambda r: -len(r[1]))
    run_lengths = [len(bs) for c, bs in core_runs[0]]
    perm = np.array(
        [[b for c, bs in rl for b in bs] for rl in core_runs], dtype=np.int64
    )
    wu_cls = np.array([[c for c, bs in rl] for rl in core_runs], dtype=np.int64)
    return run_lengths, perm, wu_cls


def _build(run_lengths, nonzero_bias: bool):
    nruns = len(run_lengths)
    xgs = _group_sizes(BLOC)
    wgs = _group_sizes(nruns)
    # batch j -> (x-group, index within group, group base batch)
    xg_of = []
    for gi, gs in enumerate(xgs):
        for jj in range(gs):
            xg_of.append((gi, jj))
    xg_base = np.cumsum([0] + xgs).tolist()
    wg_of = []
    for gi, gs in enumerate(wgs):
        for ss in range(gs):
            wg_of.append((gi, ss))
    wg_base = np.cumsum([0] + wgs).tolist()

    nc = bacc.Bacc(
        "TRN2", target_bir_lowering=False, debug=False, num_devices=NCORES
    )
    xt_d = nc.dram_tensor("xt", [128, BLOC * XB], BF16, kind="ExternalInput")
    wu_d = nc.dram_tensor("wu", [128, nruns * WB], BF16, kind="ExternalInput")
    bias_d = (
        nc.dram_tensor("bias", [1, DOUT], F32, kind="ExternalInput")
        if nonzero_bias
        else None
    )
    out_d = nc.dram_tensor("out", [128, BLOC * OB], BF16, kind="ExternalOutput")

    with tile.TileContext(nc) as tc:
        with (
            tc.tile_pool(name="xp", bufs=5) as xp,
            tc.tile_pool(name="io", bufs=3) as io,
            tc.tile_pool(name="wp", bufs=3) as wp,
            tc.tile_pool(name="psum", bufs=8, space=bass.MemorySpace.PSUM) as psum,
            tc.tile_pool(name="one", bufs=1) as one,
        ):
            bias_t = None
            if nonzero_bias:
                bias_t = one.tile([128, TCH, DOUT], F32, tag="bias")
                bap = bias_d.ap()
                nc.sync.dma_start(
                    out=bias_t[:],
                    in_=bass.AP(tensor=bap.tensor, offset=bap.offset,
                                ap=[[0, 128], [0, TCH], bap.ap[1]]),
                )

            xt_t = None
            out_t = None
            wu_t = None
            out_g = -1
            bp = 0
            for r, L in enumerate(run_lengths):
                wg, sw = wg_of[r]
                if sw == 0:
                    gs = wgs[wg]
                    wu_t = wp.tile([128, gs, KC, DOUT], BF16, tag="wu")
                    nc.scalar.dma_start(
                        out=wu_t[:],
                        in_=wu_d[:, wg_base[wg] * WB : wg_base[wg + 1] * WB],
                    )
                for i in range(L):
                    j = bp + i
                    g, jj = xg_of[j]
                    if jj == 0:
                        if out_t is not None:
                            nc.gpsimd.dma_start(
                                out=out_d[
                                    :, xg_base[out_g] * OB : xg_base[out_g + 1] * OB
                                ],
                                in_=out_t[:],
                            )
                        gs = xgs[g]
                        xt_t = xp.tile([128, gs, KC, NTOK], BF16, tag="xt")
                        nc.sync.dma_start(
                            out=xt_t[:],
                            in_=xt_d[:, xg_base[g] * XB : xg_base[g + 1] * XB],
                        )
                        out_t = io.tile([128, gs, TCH, DOUT], BF16, tag="out")
                        out_g = g
                    ps = psum.tile([128, TCH, DOUT], F32, tag="ps")
                    for t in range(TCH):
                        for k in range(KC):
                            nc.tensor.matmul(
                                ps[:, t, :],
                                xt_t[:, jj, k, t * 128 : (t + 1) * 128],
                                wu_t[:, sw, k, :],
                                start=(k == 0),
                                stop=(k == KC - 1),
                            )
                    if nonzero_bias:
                        tmp = io.tile([128, TCH, DOUT], F32, tag="tmp")
                        nc.vector.scalar_tensor_tensor(
                            out=tmp[:], in0=ps[:, :, :], scalar=1.0,
                            in1=bias_t[:],
                            op0=mybir.AluOpType.mult, op1=mybir.AluOpType.add,
                        )
                        nc.vector.scalar_tensor_tensor(
                            out=out_t[:, jj, :, :], in0=tmp[:],
                            scalar=NEG_SLOPE, in1=tmp[:],
                            op0=mybir.AluOpType.mult, op1=mybir.AluOpType.max,
                        )
                    else:
                        nc.scalar.activation(
                            out=out_t[:, jj, :, :], in_=ps[:, :, :],
                            func=mybir.ActivationFunctionType.Copy,
                        )
                        nc.vector.scalar_tensor_tensor(
                            out=out_t[:, jj, :, :], in0=out_t[:, jj, :, :],
                            scalar=NEG_SLOPE, in1=out_t[:, jj, :, :],
                            op0=mybir.AluOpType.mult, op1=mybir.AluOpType.max,
                        )
                bp += L
            assert bp == BLOC
            nc.gpsimd.dma_start(
                out=out_d[:, xg_base[out_g] * OB : xg_base[out_g + 1] * OB],
                in_=out_t[:],
            )
    nc.compile()
    return nc


def kernel(x, indices, weight, bias, _trace=False):
    x = np.asarray(x)
    indices = np.asarray(indices).astype(np.int64)
    weight = np.asarray(weight)
    bias = np.asarray(bias)

    run_lengths, perm, wu_cls = _schedule(indices)
    nruns = len(run_lengths)

    # x[b, n, i] at permuted b, i=(k,p)  ->  xt[c][p, j, k, n]  (partition-major)
    xb = x.astype(ml_dtypes.bfloat16)
    xt = np.ascontiguousarray(
        xb[perm.reshape(-1)]
        .reshape(NCORES, BLOC, NTOK, KC, 128)
        .transpose(0, 4, 1, 3, 2)
    ).reshape(NCORES, 128, BLOC * XB)
    # weight[cls, i, o], i=(k,p) at per-run classes -> wu[c][p, r, k, o]
    wb = weight.astype(ml_dtypes.bfloat16).reshape(C, KC, 128, DOUT)
    wu = np.ascontiguousarray(
        wb[wu_cls.reshape(-1)]
        .reshape(NCORES, nruns, KC, 128, DOUT)
        .transpose(0, 3, 1, 2, 4)
    ).reshape(NCORES, 128, nruns * WB)

    nonzero_bias = bool(np.any(bias))
    key = (tuple(run_lengths), nonzero_bias)
    nc = _CACHE.get(key)
    if nc is None:
        nc = _build(run_lengths, nonzero_bias)
        _CACHE.clear()
        _CACHE[key] = nc

    in_maps = []
    for c in range(NCORES):
        m = {"xt": xt[c], "wu": wu[c]}
        if nonzero_bias:
            m["bias"] = np.ascontiguousarray(
                bias.reshape(1, DOUT).astype(np.float32)
            )
        in_maps.append(m)

    res = run_bass_kernel_spmd(
        nc, in_maps, core_ids=list(range(NCORES)), trace=_trace
    )
    LAST["results"] = res
    LAST["nruns"] = nruns

    # out[c][p, j, t, o] -> out[perm[c, j], n=(t,p), o]
    full = np.empty((B, NTOK, DOUT), dtype=np.float32)
    for c in range(NCORES):
        o = np.asarray(res.results[c]["out"]).reshape(128, BLOC, TCH, DOUT)
        o = o.transpose(1, 2, 0, 3).reshape(BLOC, NTOK, DOUT).astype(np.float32)
        full[perm[c]] = o
    return full


# revision 25
# speedup vs baseline: 1.0890x; 1.0095x over previous
"""Trainium2 Bass kernel for AdaptiveReLULayer (MoE-style routed batched matmul).

    out[b] = LeakyReLU_0.2(x[b] @ weight[indices[b]] + bias)
    x: [2048, 256, 256] f32, indices: [2048] int, weight: [1024, 256, 256] f32

Strategy: data parallelism over the batch dim B=2048 across 8 NeuronCores
(256 batches/core), with an index-aware schedule: batches that share a weight
index are assigned to the same core as a "run" (length 1..4), so each run's
weight tile is DMA'd from HBM once and reused from SBUF (~47% less weight
traffic).  Run-length COUNTS are equalized across cores by splitting runs, so
all 8 cores execute the same static SPMD graph; only the data differs.

The weight gather is resolved on the host while sharding (pure addressing),
and x is pre-permuted/transposed on the host into a partition-major layout,
so every device DMA is one fully-contiguous strided block (16KB/partition
packets).  Group sizes taper at both ends (4,8,...16...,8,4) so the
first matmul starts after ~0.5MiB of loads and the final store is tiny.
Compute is bf16 with fp32 PSUM accumulation (~3e-3 relative error).
LeakyReLU: ACT copies the 512-wide PSUM bank to SBUF (bf16), then one DVE
scalar_tensor_tensor computes (v*0.2) max v in place.
"""

import numpy as np
import ml_dtypes

import concourse.bass as bass
import concourse.tile as tile
import concourse.mybir as mybir
from concourse import bacc
from concourse.bass_utils import run_bass_kernel_spmd

B, NTOK, DIN, DOUT, C = 2048, 256, 256, 256, 1024
NCORES = 8
BLOC = B // NCORES          # 256 batches per core
KC = DIN // 128             # contraction chunks of 128
TCH = NTOK // 128           # token chunks of 128
MAXRUN = 4
NEG_SLOPE = 0.2
BF16 = mybir.dt.bfloat16
F32 = mybir.dt.float32

XB = KC * NTOK              # free elems per batch of xt, per partition
WB = KC * DOUT              # free elems per run of wu, per partition
OB = TCH * DOUT             # free elems per batch of out, per partition

LAST = {}                   # stash of the last run's BassKernelResults
_CACHE = {}                 # compiled graph cache keyed by run structure


def _group_sizes(total, taper=(4, 8), mid=16):
    """Tapered group sizes: small at both ends, `mid` in the middle."""
    head = []
    need = total
    for t in taper:
        if need - t < sum(taper):
            break
        head.append(t)
        need -= t
    tail = []
    for t in taper:
        if need - t <= 0:
            break
        tail.append(t)
        need -= t
    assert need >= 0
    mids, rem = divmod(need, mid)
    out = head + [mid] * mids + ([rem] if rem else []) + tail[::-1]
    assert sum(out) == total, (out, total)
    return out


def _schedule(indices):
    """Partition the 2048 batches into 8 cores of 256 as runs of equal-index
    batches (length 1..MAXRUN).  Returns (run_lengths, perm, wu_cls): one
    shared run-length list (identical for all cores), perm [NCORES, BLOC]
    global batch ids in processing order, wu_cls [NCORES, NRUNS] weight class
    per run.
    """
    by_cls = {}
    for b, c in enumerate(indices.tolist()):
        by_cls.setdefault(c, []).append(b)

    runs = []
    for c, bs in by_cls.items():
        for i in range(0, len(bs), MAXRUN):
            runs.append((c, bs[i : i + MAXRUN]))
    runs.sort(key=lambda r: -len(r[1]))

    caps = [BLOC] * NCORES
    core_runs = [[] for _ in range(NCORES)]
    for c, bs in runs:
        while bs:
            k = int(np.argmax(caps))
            take = min(len(bs), caps[k])
            assert take > 0
            core_runs[k].append((c, bs[:take]))
            caps[k] -= take
            bs = bs[take:]
    assert all(v == 0 for v in caps)

    def counts(rl):
        n = [0] * (MAXRUN + 1)
        for c, bs in rl:
            n[len(bs)] += 1
        return n

    for L in range(MAXRUN, 1, -1):
        tgt = min(counts(rl)[L] for rl in core_runs)
        for rl in core_runs:
            while counts(rl)[L] > tgt:
                i = next(i for i, r in enumerate(rl) if len(r[1]) == L)
                c, bs = rl.pop(i)
                h = L // 2
                rl.append((c, bs[:h]))
                rl.append((c, bs[h:]))

    cn = counts(core_runs[0])
    assert all(counts(rl) == cn for rl in core_runs)

    for rl in core_runs:
        rl.sort(key=lambda r: -len(r[1]))
    run_lengths = [len(bs) for c, bs in core_runs[0]]
    perm = np.array(
        [[b for c, bs in rl for b in bs] for rl in core_runs], dtype=np.int64
    )
    wu_cls = np.array([[c for c, bs in rl] for rl in core_runs], dtype=np.int64)
    return run_lengths, perm, wu_cls


def _build(run_lengths, nonzero_bias: bool):
    nruns = len(run_lengths)
    xgs = _group_sizes(BLOC)
    wgs = _group_sizes(nruns)
    # batch j -> (x-group, index within group, group base batch)
    xg_of = []
    for gi, gs in enumerate(xgs):
        for jj in range(gs):
            xg_of.append((gi, jj))
    xg_base = np.cumsum([0] + xgs).tolist()
    wg_of = []
    for gi, gs in enumerate(wgs):
        for ss in range(gs):
            wg_of.append((gi, ss))
    wg_base = np.cumsum([0] + wgs).tolist()

    nc = bacc.Bacc(
        "TRN2", target_bir_lowering=False, debug=False, num_devices=NCORES
    )
    xt_d = nc.dram_tensor("xt", [128, BLOC * XB], BF16, kind="ExternalInput")
    wu_d = nc.dram_tensor("wu", [128, nruns * WB], BF16, kind="ExternalInput")
    bias_d = (
        nc.dram_tensor("bias", [1, DOUT], F32, kind="ExternalInput")
        if nonzero_bias
        else None
    )
    out_d = nc.dram_tensor("out", [128, BLOC * OB], BF16, kind="ExternalOutput")

    with tile.TileContext(nc) as tc:
        with (
            tc.tile_pool(name="xp", bufs=5) as xp,
            tc.tile_pool(name="io", bufs=3) as io,
            tc.tile_pool(name="wp", bufs=3) as wp,
            tc.tile_pool(name="psum", bufs=8, space=bass.MemorySpace.PSUM) as psum,
            tc.tile_pool(name="one", bufs=1) as one,
        ):
            bias_t = None
            if nonzero_bias:
                bias_t = one.tile([128, TCH, DOUT], F32, tag="bias")
                bap = bias_d.ap()
                nc.sync.dma_start(
                    out=bias_t[:],
                    in_=bass.AP(tensor=bap.tensor, offset=bap.offset,
                                ap=[[0, 128], [0, TCH], bap.ap[1]]),
                )

            xt_t = None
            out_t = None
            wu_t = None
            out_g = -1
            bp = 0
            for r, L in enumerate(run_lengths):
                wg, sw = wg_of[r]
                if sw == 0:
                    gs = wgs[wg]
                    wu_t = wp.tile([128, gs, KC, DOUT], BF16, tag="wu")
                    nc.scalar.dma_start(
                        out=wu_t[:],
                        in_=wu_d[:, wg_base[wg] * WB : wg_base[wg + 1] * WB],
                    )
                for i in range(L):
                    j = bp + i
                    g, jj = xg_of[j]
                    if jj == 0:
                        if out_t is not None:
                            nc.gpsimd.dma_start(
                                out=out_d[
                                    :, xg_base[out_g] * OB : xg_base[out_g + 1] * OB
                                ],
                                in_=out_t[:],
                            )
                        gs = xgs[g]
                        xt_t = xp.tile([128, gs, KC, NTOK], BF16, tag="xt")
                        nc.sync.dma_start(
                            out=xt_t[:],
                            in_=xt_d[:, xg_base[g] * XB : xg_base[g + 1] * XB],
                        )
                        out_t = io.tile([128, gs, TCH, DOUT], BF16, tag="out")
                        out_g = g
                    ps = psum.tile([128, TCH, DOUT], F32, tag="ps")
                    for t in range(TCH):
                        for k in range(KC):
                            nc.tensor.matmul(
                                ps[:, t, :],
                                xt_t[:, jj, k, t * 128 : (t + 1) * 128],
                                wu_t[:, sw, k, :],
                                start=(k == 0),
                                stop=(k == KC - 1),
                            )
                    if nonzero_bias:
                        tmp = io.tile([128, TCH, DOUT], F32, tag="tmp")
                        nc.vector.scalar_tensor_tensor(
                            out=tmp[:], in0=ps[:, :, :], scalar=1.0,
                            in1=bias_t[:],
                            op0=mybir.AluOpType.mult, op1=mybir.AluOpType.add,
                        )
                        nc.vector.scalar_tensor_tensor(
                            out=out_t[:, jj, :, :], in0=tmp[:],
                            scalar=NEG_SLOPE, in1=tmp[:],
                            op0=mybir.AluOpType.mult, op1=mybir.AluOpType.max,
                        )
                    else:
                        nc.scalar.activation(
                            out=out_t[:, jj, :, :], in_=ps[:, :, :],
                            func=mybir.ActivationFunctionType.Copy,
                        )
                        nc.vector.scalar_tensor_tensor(
                            out=out_t[:, jj, :, :], in0=out_t[:, jj, :, :],
                            scalar=NEG_SLOPE, in1=out_t[:, jj, :, :],
                            op0=mybir.AluOpType.mult, op1=mybir.AluOpType.max,
                        )
                bp += L
            assert bp == BLOC
            nc.gpsimd.dma_start(
                out=out_d[:, xg_base[out_g] * OB : xg_base[out_g + 1] * OB],
                in_=out_t[:],
            )
    nc.compile()
    return nc


def kernel(x, indices, weight, bias, _trace=False):
    x = np.asarray(x)
    indices = np.asarray(indices).astype(np.int64)
    weight = np.asarray(weight)
    bias = np.asarray(bias)

    run_lengths, perm, wu_cls = _schedule(indices)
    nruns = len(run_lengths)

    # x[b, n, i] at permuted b, i=(k,p)  ->  xt[c][p, j, k, n]  (partition-major)
    xb = x.astype(ml_dtypes.bfloat16)
    xt = np.ascontiguousarray(
        xb[perm.reshape(-1)]
        .reshape(NCORES, BLOC, NTOK, KC, 128)
        .transpose(0, 4, 1, 3, 2)
    ).reshape(NCORES, 128, BLOC * XB)
    # weight[cls, i, o], i=(k,p) at per-run classes -> wu[c][p, r, k, o]
    wb = weight.astype(ml_dtypes.bfloat16).reshape(C, KC, 128, DOUT)
    wu = np.ascontiguousarray(
        wb[wu_cls.reshape(-1)]
        .reshape(NCORES, nruns, KC, 128, DOUT)
        .transpose(0, 3, 1, 2, 4)
    ).reshape(NCORES, 128, nruns * WB)

    nonzero_bias = bool(np.any(bias))
    key = (tuple(run_lengths), nonzero_bias)
    nc = _CACHE.get(key)
    if nc is None:
        nc = _build(run_lengths, nonzero_bias)
        _CACHE.clear()
        _CACHE[key] = nc

    in_maps = []
    for c in range(NCORES):
        m = {"xt": xt[c], "wu": wu[c]}
        if nonzero_bias:
            m["bias"] = np.ascontiguousarray(
                bias.reshape(1, DOUT).astype(np.float32)
            )
        in_maps.append(m)

    res = run_bass_kernel_spmd(
        nc, in_maps, core_ids=list(range(NCORES)), trace=_trace
    )
    LAST["results"] = res
    LAST["nruns"] = nruns

    # out[c][p, j, t, o] -> out[perm[c, j], n=(t,p), o]
    full = np.empty((B, NTOK, DOUT), dtype=np.float32)
    for c in range(NCORES):
        o = np.asarray(res.results[c]["out"]).reshape(128, BLOC, TCH, DOUT)
        o = o.transpose(1, 2, 0, 3).reshape(BLOC, NTOK, DOUT).astype(np.float32)
        full[perm[c]] = o
    return full
